# revision 1
# baseline (speedup 1.0000x reference)
"""Trainium2 Bass kernel for DigitConvolutionalModel.

Model: x[B,784] -> reshape 28x28 -> 3x3 valid conv -> [B,676] -> FC(676,300)
       -> ReLU -> FC(300,10).

Strategy:
  * Fold the conv into FC1 on the host: feat @ w1 == x @ W1e where
    W1e[784,300] = C @ w1 (C = sparse conv scatter). Weight-only preprocessing.
  * Pure data parallel over 8 NeuronCores: batch shard of 8192 rows per core.
  * Per-core shard is passed pre-transposed (feature-major) so the contraction
    dim (784 = 7 chunks x 112) sits on SBUF partitions; the kernel computes
    transposed activations throughout (batch on the free axis):
        a1T[300,b] = relu(W1e.T @ xT + b1);  yT[10,b] = w2.T @ a1T + b2
    Biases are per-partition -> fold into scalar-engine activation bias.
  * float32r matmul operands: full-rate PE streaming with fp32 PSUM
    accumulation (~2e-4 rel err vs the fp32 reference).
  * Output yT[10,8192] per core, un-transposed/gathered on host.
"""

import os
import sys

sys.path.insert(0, "/opt/trn_rl_repo")

import numpy as np

import concourse.tile as tile
from concourse import bacc, mybir
from concourse.bass_utils import run_bass_kernel_spmd

# ---- problem constants (hardcoded per harness contract) ----
B = 65536
D = 784  # 28*28
H = 300
O = 10
IMG = 28
KH = KW = 3
OUT_HW = IMG - KH + 1  # 26

N_CORES = 8
BS = B // N_CORES  # 8192 rows per core

KCH = 7  # contraction chunks
KP = D // KCH  # 112 partitions per chunk
BT = int(os.environ.get("BT_SIZE", "512"))  # batch tile (512 = one PSUM bank)
NBT = BS // BT
MPAD = 128  # padded partition count for hidden-chunk tensors

# matmul operand dtype: f32 (exact) | f32r (fast fp32 mode) | f16 (half inputs)
_MM_CHOICE = os.environ.get("BASS_MM_DT", "f32r")
MM_DT = {"f32": mybir.dt.float32, "f16": mybir.dt.float16}.get(
    _MM_CHOICE, mybir.dt.float32r
)
MM_NP = np.float16 if _MM_CHOICE == "f16" else np.float32

# hidden-dim chunking (sum must be H)
if os.environ.get("M_UNEVEN", "0") == "1":
    M_CHUNKS = [128, 128, 44]
else:
    M_CHUNKS = [100, 100, 100]
M_OFFS = [sum(M_CHUNKS[:i]) for i in range(len(M_CHUNKS))]
MCH = len(M_CHUNKS)

# tunables (env-overridable for experiments)
SUBT_DEFAULT = int(os.environ.get("SUBT", "1"))
XP_BUFS = int(os.environ.get("XP_BUFS", "3" if SUBT_DEFAULT <= 1 else "1"))
AP_BUFS = int(os.environ.get("AP_BUFS", "3" if SUBT_DEFAULT <= 1 else "2"))
PS1_BUFS = int(os.environ.get("PS1_BUFS", ("4" if BT <= 512 else "3") if SUBT_DEFAULT <= 1 else "2"))
PS2_BUFS = int(os.environ.get("PS2_BUFS", "2" if BT <= 512 else "1"))
X_DMA_SPLIT = int(os.environ.get("X_DMA_SPLIT", "1"))  # k-chunk granularity of x loads
X_LAYOUT = os.environ.get("X_LAYOUT", "bt")  # "bt": batch-tile-major (contiguous loads); "k": k-major
REPS = int(os.environ.get("KERNEL_REPS", "1"))  # timing only: repeat body in-module
SUBT = int(os.environ.get("SUBT", "1"))  # batch subtiles sharing one weight load

_cache = {}


def _build_nc():
    f32 = mybir.dt.float32
    mdt = MM_DT

    nc = bacc.Bacc("TRN2", target_bir_lowering=False, debug=False, num_devices=N_CORES)
    if X_LAYOUT == "bt":
        xt_d = nc.declare_dram_parameter("xt", [KP, NBT, KCH, BT], mdt, isOutput=False)
    else:
        xt_d = nc.declare_dram_parameter("xt", [KP, KCH, BS], mdt, isOutput=False)
    w1_d = nc.declare_dram_parameter("w1e", [KP, KCH * H], mdt, isOutput=False)
    b1_d = nc.declare_dram_parameter("b1r", [MPAD, MCH], f32, isOutput=False)
    w2_d = nc.declare_dram_parameter("w2r", [MPAD, MCH * O], mdt, isOutput=False)
    b2_d = nc.declare_dram_parameter("b2r", [O, 1], f32, isOutput=False)
    yt_d = nc.declare_dram_parameter("yt", [O, BS], f32, isOutput=True)

    with tile.TileContext(nc) as tc:
        with (
            tc.tile_pool(name="singles", bufs=1) as singles,
            tc.tile_pool(name="xp", bufs=XP_BUFS) as xp,
            tc.tile_pool(name="ap", bufs=AP_BUFS) as ap,
            tc.tile_pool(name="yp", bufs=3) as yp,
            tc.tile_pool(name="ps1", bufs=PS1_BUFS, space="PSUM") as ps1p,
            tc.tile_pool(name="ps2", bufs=PS2_BUFS, space="PSUM") as ps2p,
        ):
            w1sb = singles.tile([KP, KCH * H], mdt)
            nc.sync.dma_start(w1sb[:], w1_d[:])
            b1sb = singles.tile([MPAD, MCH], f32)
            nc.sync.dma_start(b1sb[:], b1_d[:])
            w2sb = singles.tile([MPAD, MCH * O], mdt)
            nc.sync.dma_start(w2sb[:], w2_d[:])
            b2sb = singles.tile([O, 1], f32)
            nc.sync.dma_start(b2sb[:], b2_d[:])

            def load_x(bt):
                tag = "xt" if SUBT <= 1 else f"xt{bt % (SUBT + 2)}"
                xt = xp.tile([KP, KCH, BT], mdt, name=tag)
                step = (KCH + X_DMA_SPLIT - 1) // X_DMA_SPLIT if X_DMA_SPLIT > 1 else KCH
                for s in range(0, KCH, step):
                    e = min(s + step, KCH)
                    if X_LAYOUT == "bt":
                        nc.sync.dma_start(xt[:, s:e, :], xt_d[:, bt, s:e, :])
                    else:
                        nc.sync.dma_start(
                            xt[:, s:e, :],
                            xt_d[:, s:e, bt * BT : (bt + 1) * BT],
                        )
                return xt

            def layer2_store(a1, bt):
                ps2 = ps2p.tile([O, BT], f32)
                for j in range(MCH):
                    mlen = M_CHUNKS[j]
                    nc.tensor.matmul(
                        ps2[:],
                        w2sb[0:mlen, j * O : (j + 1) * O],
                        a1[0:mlen, j, :],
                        start=(j == 0),
                        stop=(j == MCH - 1),
                    )
                yt = yp.tile([O, BT], f32)
                nc.vector.tensor_scalar_add(yt[:], ps2[:], b2sb[:, 0:1])
                nc.sync.dma_start(yt_d[:, bt * BT : (bt + 1) * BT], yt[:])

            if SUBT <= 1:
                for bt in [i for _ in range(REPS) for i in range(NBT)]:
                    xt = load_x(bt)
                    a1 = ap.tile([MPAD, MCH, BT], mdt)
                    for j in range(MCH):
                        mlen, moff = M_CHUNKS[j], M_OFFS[j]
                        ps = ps1p.tile([MPAD, BT], f32)
                        for k in range(KCH):
                            nc.tensor.matmul(
                                ps[0:mlen, :],
                                w1sb[:, k * H + moff : k * H + moff + mlen],
                                xt[:, k, :],
                                start=(k == 0),
                                stop=(k == KCH - 1),
                            )
                        nc.scalar.activation(
                            a1[0:mlen, j, :],
                            ps[0:mlen, :],
                            mybir.ActivationFunctionType.Relu,
                            bias=b1sb[0:mlen, j : j + 1],
                        )
                    layer2_store(a1, bt)
            else:
                # weight-reuse grouping: SUBT batch subtiles per (j,k) stationary
                for g in [i for _ in range(REPS) for i in range(NBT // SUBT)]:
                    bts = [g * SUBT + s for s in range(SUBT)]
                    xts = [load_x(bt) for bt in bts]
                    a1s = [
                        ap.tile([MPAD, MCH, BT], mdt, name=f"a1{s}")
                        for s in range(SUBT)
                    ]
                    for j in range(MCH):
                        mlen, moff = M_CHUNKS[j], M_OFFS[j]
                        pss = [
                            ps1p.tile([MPAD, BT], f32, name=f"ps{s}")
                            for s in range(SUBT)
                        ]
                        for k in range(KCH):
                            for s in range(SUBT):
                                nc.tensor.matmul(
                                    pss[s][0:mlen, :],
                                    w1sb[:, k * H + moff : k * H + moff + mlen],
                                    xts[s][:, k, :],
                                    start=(k == 0),
                                    stop=(k == KCH - 1),
                                )
                        for s in range(SUBT):
                            nc.scalar.activation(
                                a1s[s][0:mlen, j, :],
                                pss[s][0:mlen, :],
                                mybir.ActivationFunctionType.Relu,
                                bias=b1sb[0:mlen, j : j + 1],
                            )
                    for s in range(SUBT):
                        layer2_store(a1s[s], bts[s])

    nc.compile()
    return nc


def _host_prep_weights(conv_w, w1, b1, w2, b2):
    # Fold conv into FC1: W1e = C @ w1, computed in f64 then cast.
    w1g = w1.astype(np.float64).reshape(OUT_HW, OUT_HW, H)
    w1e = np.zeros((IMG, IMG, H), dtype=np.float64)
    cw = conv_w.astype(np.float64)
    for di in range(KH):
        for dj in range(KW):
            w1e[di : di + OUT_HW, dj : dj + OUT_HW, :] += cw[di, dj] * w1g
    w1e = w1e.reshape(D, H).astype(np.float32)

    w1e_r = np.ascontiguousarray(
        w1e.reshape(KCH, KP, H).transpose(1, 0, 2).reshape(KP, KCH * H)
    ).astype(MM_NP)
    b1f = b1.reshape(H)
    b1_r = np.zeros((MPAD, MCH), np.float32)
    w2_r = np.zeros((MPAD, MCH * O), MM_NP)
    for j in range(MCH):
        mlen, moff = M_CHUNKS[j], M_OFFS[j]
        b1_r[0:mlen, j] = b1f[moff : moff + mlen]
        w2_r[0:mlen, j * O : (j + 1) * O] = w2[moff : moff + mlen, :]
    b2_r = np.ascontiguousarray(b2.reshape(O, 1))
    return w1e_r, b1_r, w2_r, b2_r


def _host_prep_x(xc):
    """Per-core shard [BS, 784] -> feature-major DRAM layout."""
    xc = xc.astype(MM_NP)
    if X_LAYOUT == "bt":
        # xt[p, bt, k, b] = xc[bt*BT + b, k*KP + p]: per-(partition, batch-tile)
        # loads are fully contiguous per partition.
        return np.ascontiguousarray(
            xc.reshape(NBT, BT, KCH, KP).transpose(3, 0, 2, 1)
        )
    # xt[p, k, b] = xc[b, k*KP + p]
    return np.ascontiguousarray(xc.T.reshape(KCH, KP, BS).transpose(1, 0, 2))


def kernel(x, conv_w, w1, b1, w2, b2):
    x = np.asarray(x, dtype=np.float32)
    w1e_r, b1_r, w2_r, b2_r = _host_prep_weights(
        np.asarray(conv_w, np.float32),
        np.asarray(w1, np.float32),
        np.asarray(b1, np.float32),
        np.asarray(w2, np.float32),
        np.asarray(b2, np.float32),
    )

    if "nc" not in _cache:
        _cache["nc"] = _build_nc()
    nc = _cache["nc"]

    in_maps = []
    for c in range(N_CORES):
        xc = x[c * BS : (c + 1) * BS]  # [BS, 784]
        xt = _host_prep_x(xc)
        in_maps.append(
            {"xt": xt, "w1e": w1e_r, "b1r": b1_r, "w2r": w2_r, "b2r": b2_r}
        )

    res = run_bass_kernel_spmd(nc, in_maps, list(range(N_CORES)))

    y = np.empty((B, O), dtype=np.float32)
    for c in range(N_CORES):
        y[c * BS : (c + 1) * BS] = res.results[c]["yt"].T
    return y



# revision 2
# speedup vs baseline: 1.0036x; 1.0036x over previous
"""Trainium2 Bass kernel for DigitConvolutionalModel.

Model: x[B,784] -> reshape 28x28 -> 3x3 valid conv -> [B,676] -> FC(676,300)
       -> ReLU -> FC(300,10).

Strategy:
  * Fold the conv into FC1 on the host: feat @ w1 == x @ W1e where
    W1e[784,300] = C @ w1 (C = sparse conv scatter). Weight-only preprocessing.
  * Pure data parallel over 8 NeuronCores: batch shard of 8192 rows per core.
  * Per-core shard is passed pre-transposed (feature-major) so the contraction
    dim (784 = 7 chunks x 112) sits on SBUF partitions; the kernel computes
    transposed activations throughout (batch on the free axis):
        a1T[300,b] = relu(W1e.T @ xT + b1);  yT[10,b] = w2.T @ a1T
  * fp16 matmul operands (1 cyc/row at every PE p-state, half the HBM
    traffic of fp32) with fp32 PSUM accumulation; biases fp32.
  * Layer 2 is column-tiled: the three K=100 hidden chunks run as
    concurrent matmuls in PE column groups (0,32,64); the three partial
    yT outputs land on disjoint PSUM partitions and are summed on the
    host during the gather step (b2 is added there too).
  * Layer 2 for tile i is emitted between L1 j-chunks of tile i+1 so the
    PE never waits on the ReLU; no PE gaps -> stays at the 2.4 GHz p-state.
  * Output: partials ytp[3,10,8192] per core, summed/transposed on host.
"""

import os
import sys

sys.path.insert(0, "/opt/trn_rl_repo")

import numpy as np

import concourse.tile as tile
from concourse import bacc, mybir
from concourse.bass_utils import run_bass_kernel_spmd

# ---- problem constants (hardcoded per harness contract) ----
B = 65536
D = 784  # 28*28
H = 300
O = 10
IMG = 28
KH = KW = 3
OUT_HW = IMG - KH + 1  # 26

N_CORES = 8
BS = B // N_CORES  # 8192 rows per core

KCH = 7  # contraction chunks
KP = D // KCH  # 112 partitions per chunk
BT = int(os.environ.get("BT_SIZE", "512"))  # batch tile (512 = one PSUM bank)
NBT = BS // BT
MPAD = 128  # padded partition count for hidden-chunk tensors

# matmul operand dtype: f16 (default) | bf16 | f32r | f32
_MM_CHOICE = os.environ.get("BASS_MM_DT", "f16")
MM_DT = {
    "f32": mybir.dt.float32,
    "f32r": mybir.dt.float32r,
    "bf16": mybir.dt.bfloat16,
}.get(_MM_CHOICE, mybir.dt.float16)
MM_NP = {
    "f32": np.float32,
    "f32r": np.float32,
    "bf16": None,  # filled below
}.get(_MM_CHOICE, np.float16)
if MM_NP is None:
    import ml_dtypes

    MM_NP = ml_dtypes.bfloat16

M_CHUNKS = [100, 100, 100]  # hidden-dim chunking (sum must be H)
M_OFFS = [sum(M_CHUNKS[:i]) for i in range(len(M_CHUNKS))]
MCH = len(M_CHUNKS)

# tunables (env-overridable for experiments)
XP_BUFS = int(os.environ.get("XP_BUFS", "3"))
AP_BUFS = int(os.environ.get("AP_BUFS", "3"))
PS1_BUFS = int(os.environ.get("PS1_BUFS", "4"))
PS2_BUFS = int(os.environ.get("PS2_BUFS", "2"))
X_DMA_SPLIT = int(os.environ.get("X_DMA_SPLIT", "1"))  # k-chunk granularity of x loads
L2_COLTILE = os.environ.get("L2_COLTILE", "1") == "1"
L2_PIPELINE = os.environ.get("L2_PIPELINE", "1") == "1"
REPS = int(os.environ.get("KERNEL_REPS", "1"))  # timing only: repeat body in-module

_cache = {}


def _build_nc():
    f32 = mybir.dt.float32
    mdt = MM_DT

    nc = bacc.Bacc("TRN2", target_bir_lowering=False, debug=False, num_devices=N_CORES)
    xt_d = nc.declare_dram_parameter("xt", [KP, NBT, KCH, BT], mdt, isOutput=False)
    w1_d = nc.declare_dram_parameter("w1e", [KP, KCH * H], mdt, isOutput=False)
    b1_d = nc.declare_dram_parameter("b1r", [MPAD, MCH], f32, isOutput=False)
    w2_d = nc.declare_dram_parameter("w2r", [MPAD, MCH * O], mdt, isOutput=False)
    ytp_d = nc.declare_dram_parameter("ytp", [MCH, O, BS], f32, isOutput=True)

    # partial-output partition bases (column groups) when col-tiling
    PB = [32 * j for j in range(MCH)] if L2_COLTILE else [0] * MCH
    YP_P = PB[-1] + O if L2_COLTILE else O  # partitions used by ps2/yt tiles

    with tile.TileContext(nc) as tc:
        with (
            tc.tile_pool(name="singles", bufs=1) as singles,
            tc.tile_pool(name="xp", bufs=XP_BUFS) as xp,
            tc.tile_pool(name="ap", bufs=AP_BUFS) as ap,
            tc.tile_pool(name="yp", bufs=3) as yp,
            tc.tile_pool(name="ps1", bufs=PS1_BUFS, space="PSUM") as ps1p,
            tc.tile_pool(name="ps2", bufs=PS2_BUFS, space="PSUM") as ps2p,
        ):
            w1sb = singles.tile([KP, KCH * H], mdt)
            nc.sync.dma_start(w1sb[:], w1_d[:])
            b1sb = singles.tile([MPAD, MCH], f32)
            nc.sync.dma_start(b1sb[:], b1_d[:])
            w2sb = singles.tile([MPAD, MCH * O], mdt)
            nc.sync.dma_start(w2sb[:], w2_d[:])

            def load_x(bt, split):
                xt = xp.tile([KP, KCH, BT], mdt)
                step = (KCH + split - 1) // split if split > 1 else KCH
                for s in range(0, KCH, step):
                    e = min(s + step, KCH)
                    nc.sync.dma_start(xt[:, s:e, :], xt_d[:, bt, s:e, :])
                return xt

            def layer2_store(a1, bt):
                ps2 = ps2p.tile([YP_P, BT], f32)
                for j in range(MCH):
                    mlen = M_CHUNKS[j]
                    if L2_COLTILE:
                        nc.tensor.matmul(
                            ps2[PB[j] : PB[j] + O, :],
                            w2sb[0:mlen, j * O : (j + 1) * O],
                            a1[0:mlen, j, :],
                            start=True,
                            stop=True,
                            tile_position=(0, PB[j]),
                        )
                    else:
                        nc.tensor.matmul(
                            ps2[:],
                            w2sb[0:mlen, j * O : (j + 1) * O],
                            a1[0:mlen, j, :],
                            start=(j == 0),
                            stop=(j == MCH - 1),
                        )
                yt = yp.tile([YP_P, BT], f32)
                nc.vector.tensor_scalar_add(yt[:], ps2[:], 0.0)
                if L2_COLTILE:
                    for j in range(MCH):
                        nc.sync.dma_start(
                            ytp_d[j, :, bt * BT : (bt + 1) * BT],
                            yt[PB[j] : PB[j] + O, :],
                        )
                else:
                    nc.sync.dma_start(ytp_d[0, :, bt * BT : (bt + 1) * BT], yt[:])

            pending = None
            for idx, bt in enumerate(i for _ in range(REPS) for i in range(NBT)):
                xt = load_x(bt, split=(KCH if idx == 0 else X_DMA_SPLIT))
                a1 = ap.tile([MPAD, MCH, BT], mdt)
                for j in range(MCH):
                    mlen, moff = M_CHUNKS[j], M_OFFS[j]
                    ps = ps1p.tile([MPAD, BT], f32)
                    for k in range(KCH):
                        nc.tensor.matmul(
                            ps[0:mlen, :],
                            w1sb[:, k * H + moff : k * H + moff + mlen],
                            xt[:, k, :],
                            start=(k == 0),
                            stop=(k == KCH - 1),
                        )
                    nc.scalar.activation(
                        a1[0:mlen, j, :],
                        ps[0:mlen, :],
                        mybir.ActivationFunctionType.Relu,
                        bias=b1sb[0:mlen, j : j + 1],
                    )
                    if j == 0 and pending is not None and L2_PIPELINE:
                        layer2_store(*pending)
                        pending = None
                if L2_PIPELINE:
                    pending = (a1, bt)
                else:
                    layer2_store(a1, bt)
            if pending is not None:
                layer2_store(*pending)

    nc.compile()
    return nc


def _host_prep_weights(conv_w, w1, b1, w2):
    # Fold conv into FC1: W1e = C @ w1, computed in f64 then cast.
    w1g = w1.astype(np.float64).reshape(OUT_HW, OUT_HW, H)
    w1e = np.zeros((IMG, IMG, H), dtype=np.float64)
    cw = conv_w.astype(np.float64)
    for di in range(KH):
        for dj in range(KW):
            w1e[di : di + OUT_HW, dj : dj + OUT_HW, :] += cw[di, dj] * w1g
    w1e = w1e.reshape(D, H).astype(np.float32)

    w1e_r = np.ascontiguousarray(
        w1e.reshape(KCH, KP, H).transpose(1, 0, 2).reshape(KP, KCH * H)
    ).astype(MM_NP)
    b1f = b1.reshape(H)
    b1_r = np.zeros((MPAD, MCH), np.float32)
    w2_r = np.zeros((MPAD, MCH * O), MM_NP)
    for j in range(MCH):
        mlen, moff = M_CHUNKS[j], M_OFFS[j]
        b1_r[0:mlen, j] = b1f[moff : moff + mlen]
        w2_r[0:mlen, j * O : (j + 1) * O] = w2[moff : moff + mlen, :]
    return w1e_r, b1_r, w2_r


def _host_prep_x(xc):
    """Per-core shard [BS, 784] -> feature-major DRAM layout.

    xt[p, bt, k, b] = xc[bt*BT + b, k*KP + p]: per-(partition, batch-tile)
    loads are fully contiguous per partition.
    """
    return np.ascontiguousarray(
        xc.astype(MM_NP).reshape(NBT, BT, KCH, KP).transpose(3, 0, 2, 1)
    )


def kernel(x, conv_w, w1, b1, w2, b2):
    x = np.asarray(x, dtype=np.float32)
    w1e_r, b1_r, w2_r = _host_prep_weights(
        np.asarray(conv_w, np.float32),
        np.asarray(w1, np.float32),
        np.asarray(b1, np.float32),
        np.asarray(w2, np.float32),
    )
    b2 = np.asarray(b2, np.float32).reshape(1, O)

    if "nc" not in _cache:
        _cache["nc"] = _build_nc()
    nc = _cache["nc"]

    in_maps = []
    for c in range(N_CORES):
        xc = x[c * BS : (c + 1) * BS]  # [BS, 784]
        in_maps.append(
            {"xt": _host_prep_x(xc), "w1e": w1e_r, "b1r": b1_r, "w2r": w2_r}
        )

    res = run_bass_kernel_spmd(nc, in_maps, list(range(N_CORES)))

    y = np.empty((B, O), dtype=np.float32)
    for c in range(N_CORES):
        ytp = res.results[c]["ytp"]  # [MCH, O, BS] partials
        yc = ytp.sum(axis=0) if L2_COLTILE else ytp[0]
        y[c * BS : (c + 1) * BS] = yc.T + b2
    return y


# revision 4
# speedup vs baseline: 1.1021x; 1.0981x over previous
"""Trainium2 Bass kernel for DigitConvolutionalModel.

Model: x[B,784] -> reshape 28x28 -> 3x3 valid conv -> [B,676] -> FC(676,300)
       -> ReLU -> FC(300,10).

Strategy:
  * Fold the conv into FC1 on the host: feat @ w1 == x @ W1e where
    W1e[784,300] = C @ w1 (C = sparse conv scatter). Weight-only preprocessing.
  * Pure data parallel over 8 NeuronCores: batch shard of 8192 rows per core.
  * Per-core shard is passed pre-transposed (feature-major) so the contraction
    dim (784 = 7 chunks x 112) sits on SBUF partitions; the kernel computes
    transposed activations throughout (batch on the free axis):
        a1T[300,b] = relu(W1e.T @ xT + b1);  yT[10,b] = w2.T @ a1T
  * fp16 matmul operands (1 cyc/row at every PE p-state, half the HBM
    traffic of fp32) with fp32 PSUM accumulation; biases fp32.
  * SUBT=2 batch tiles are processed per weight-stationary step: each
    LDWEIGHTS serves SUBT matmuls (measured ~25 ns/MM less PE time than
    reloading weights per matmul).
  * Layer 2 is column-tiled: the three K=100 hidden chunks run as
    concurrent matmuls in PE column groups (0,32,64); the three partial
    yT outputs land on disjoint PSUM partitions (0-9, 32-41, 64-73), are
    copied to SBUF as one [74,BT] tile, stored with one DMA, and summed
    on the host during the gather step (b2 is added there too).
  * Layer 2 for a group is emitted between L1 j-chunks of the next group
    so the PE never waits on the ReLU (no PE gaps, stays at max p-state).
  * Output: ytp[74,8192] per core; host takes rows {0-9,32-41,64-73}.
"""

import os
import sys

sys.path.insert(0, "/opt/trn_rl_repo")

import numpy as np

import concourse.tile as tile
from concourse import bacc, mybir
from concourse.bass_utils import run_bass_kernel_spmd

# ---- problem constants (hardcoded per harness contract) ----
B = 65536
D = 784  # 28*28
H = 300
O = 10
IMG = 28
KH = KW = 3
OUT_HW = IMG - KH + 1  # 26

N_CORES = 8
BS = B // N_CORES  # 8192 rows per core

KCH = 7  # contraction chunks
KP = D // KCH  # 112 partitions per chunk
BT = int(os.environ.get("BT_SIZE", "512"))  # batch tile (512 = one PSUM bank)
NBT = BS // BT
MPAD = 128  # padded partition count for hidden-chunk tensors

# matmul operand dtype: f16 (default) | bf16 | f32r | f32
_MM_CHOICE = os.environ.get("BASS_MM_DT", "f16")
MM_DT = {
    "f32": mybir.dt.float32,
    "f32r": mybir.dt.float32r,
    "bf16": mybir.dt.bfloat16,
}.get(_MM_CHOICE, mybir.dt.float16)
if _MM_CHOICE in ("f32", "f32r"):
    MM_NP = np.float32
elif _MM_CHOICE == "bf16":
    import ml_dtypes

    MM_NP = ml_dtypes.bfloat16
else:
    MM_NP = np.float16

# hidden-dim chunking (sum must be H)
M_CHUNKS = {
    "even": [100, 100, 100],
    "fwl": [128, 128, 44],
}[os.environ.get("M_CHUNKS", "even")]
M_OFFS = [sum(M_CHUNKS[:i]) for i in range(len(M_CHUNKS))]
MCH = len(M_CHUNKS)

# tunables (env-overridable for experiments)
SUBT = int(os.environ.get("SUBT", "2"))  # batch tiles per weight-stationary step
XP_BUFS = int(os.environ.get("XP_BUFS", "3"))
AP_BUFS = int(os.environ.get("AP_BUFS", "3"))
PS1_BUFS = int(os.environ.get("PS1_BUFS", "3" if SUBT == 2 else "4"))
PS2_BUFS = int(os.environ.get("PS2_BUFS", "2"))
X_DMA_SPLIT = int(os.environ.get("X_DMA_SPLIT", "1"))  # k-chunk granularity of x loads
L2_COLTILE = os.environ.get("L2_COLTILE", "1") == "1"
L2_PIPELINE = os.environ.get("L2_PIPELINE", "1") == "1"
REPS = int(os.environ.get("KERNEL_REPS", "1"))  # timing only: repeat body in-module

# partial-output partition bases (column groups) when col-tiling
PB = [32 * j for j in range(MCH)] if L2_COLTILE else [0] * MCH
YP_P = PB[-1] + O if L2_COLTILE else O  # partitions used by ps2/yt tiles

_cache = {}


def _build_nc():
    f32 = mybir.dt.float32
    mdt = MM_DT

    nc = bacc.Bacc("TRN2", target_bir_lowering=False, debug=False, num_devices=N_CORES)
    xt_d = nc.declare_dram_parameter("xt", [KP, NBT, KCH, BT], mdt, isOutput=False)
    w1_d = nc.declare_dram_parameter("w1e", [KP, KCH * H], mdt, isOutput=False)
    b1_d = nc.declare_dram_parameter("b1r", [MPAD, MCH], f32, isOutput=False)
    w2_d = nc.declare_dram_parameter("w2r", [MPAD, MCH * O], mdt, isOutput=False)
    ytp_d = nc.declare_dram_parameter("ytp", [YP_P, BS], f32, isOutput=True)

    with tile.TileContext(nc) as tc:
        with (
            tc.tile_pool(name="singles", bufs=1) as singles,
            tc.tile_pool(name="xp", bufs=XP_BUFS) as xp,
            tc.tile_pool(name="ap", bufs=AP_BUFS) as ap,
            tc.tile_pool(name="yp", bufs=3) as yp,
            tc.tile_pool(name="ps1", bufs=PS1_BUFS, space="PSUM") as ps1p,
            tc.tile_pool(name="ps2", bufs=PS2_BUFS, space="PSUM") as ps2p,
        ):
            w1sb = singles.tile([KP, KCH * H], mdt)
            nc.sync.dma_start(w1sb[:], w1_d[:])
            b1sb = singles.tile([MPAD, MCH], f32)
            nc.sync.dma_start(b1sb[:], b1_d[:])
            w2sb = singles.tile([MPAD, MCH * O], mdt)
            nc.sync.dma_start(w2sb[:], w2_d[:])

            def load_x(bt, s, split):
                xt = xp.tile([KP, KCH, BT], mdt, name=f"xt{s}")
                step = (KCH + split - 1) // split if split > 1 else KCH
                for lo in range(0, KCH, step):
                    hi = min(lo + step, KCH)
                    nc.sync.dma_start(xt[:, lo:hi, :], xt_d[:, bt, lo:hi, :])
                return xt

            def layer2_store(a1, bt):
                ps2 = ps2p.tile([YP_P, BT], f32)
                for j in range(MCH):
                    mlen = M_CHUNKS[j]
                    if L2_COLTILE:
                        nc.tensor.matmul(
                            ps2[PB[j] : PB[j] + O, :],
                            w2sb[0:mlen, j * O : (j + 1) * O],
                            a1[0:mlen, j, :],
                            start=True,
                            stop=True,
                            tile_position=(0, PB[j]),
                        )
                    else:
                        nc.tensor.matmul(
                            ps2[:],
                            w2sb[0:mlen, j * O : (j + 1) * O],
                            a1[0:mlen, j, :],
                            start=(j == 0),
                            stop=(j == MCH - 1),
                        )
                yt = yp.tile([YP_P, BT], f32)
                nc.vector.tensor_scalar_add(yt[:], ps2[:], 0.0)
                nc.sync.dma_start(ytp_d[:, bt * BT : (bt + 1) * BT], yt[:])

            order = [i for _ in range(REPS) for i in range(NBT)]
            groups = [order[i : i + SUBT] for i in range(0, len(order), SUBT)]
            pending = []
            for gi, bts in enumerate(groups):
                xts = [
                    load_x(bt, s, split=(KCH if gi == 0 else X_DMA_SPLIT))
                    for s, bt in enumerate(bts)
                ]
                a1s = [
                    ap.tile([MPAD, MCH, BT], mdt, name=f"a1{s}")
                    for s in range(len(bts))
                ]
                for j in range(MCH):
                    mlen, moff = M_CHUNKS[j], M_OFFS[j]
                    pss = [
                        ps1p.tile([MPAD, BT], f32, name=f"ps{s}")
                        for s in range(len(bts))
                    ]
                    for k in range(KCH):
                        for s in range(len(bts)):
                            nc.tensor.matmul(
                                pss[s][0:mlen, :],
                                w1sb[:, k * H + moff : k * H + moff + mlen],
                                xts[s][:, k, :],
                                start=(k == 0),
                                stop=(k == KCH - 1),
                            )
                    for s in range(len(bts)):
                        nc.scalar.activation(
                            a1s[s][0:mlen, j, :],
                            pss[s][0:mlen, :],
                            mybir.ActivationFunctionType.Relu,
                            bias=b1sb[0:mlen, j : j + 1],
                        )
                    if j == 0 and pending and L2_PIPELINE:
                        for p in pending:
                            layer2_store(*p)
                        pending = []
                if L2_PIPELINE:
                    pending = [(a1s[s], bts[s]) for s in range(len(bts))]
                else:
                    for s in range(len(bts)):
                        layer2_store(a1s[s], bts[s])
            for p in pending:
                layer2_store(*p)

    nc.compile()
    return nc


def _host_prep_weights(conv_w, w1, b1, w2):
    # Fold conv into FC1: W1e = C @ w1, computed in f64 then cast.
    w1g = w1.astype(np.float64).reshape(OUT_HW, OUT_HW, H)
    w1e = np.zeros((IMG, IMG, H), dtype=np.float64)
    cw = conv_w.astype(np.float64)
    for di in range(KH):
        for dj in range(KW):
            w1e[di : di + OUT_HW, dj : dj + OUT_HW, :] += cw[di, dj] * w1g
    w1e = w1e.reshape(D, H).astype(np.float32)

    w1e_r = np.ascontiguousarray(
        w1e.reshape(KCH, KP, H).transpose(1, 0, 2).reshape(KP, KCH * H)
    ).astype(MM_NP)
    b1f = b1.reshape(H)
    b1_r = np.zeros((MPAD, MCH), np.float32)
    w2_r = np.zeros((MPAD, MCH * O), MM_NP)
    for j in range(MCH):
        mlen, moff = M_CHUNKS[j], M_OFFS[j]
        b1_r[0:mlen, j] = b1f[moff : moff + mlen]
        w2_r[0:mlen, j * O : (j + 1) * O] = w2[moff : moff + mlen, :]
    return w1e_r, b1_r, w2_r


def _host_prep_x(xc):
    """Per-core shard [BS, 784] -> feature-major DRAM layout.

    xt[p, bt, k, b] = xc[bt*BT + b, k*KP + p]: per-(partition, batch-tile)
    loads are fully contiguous per partition.
    """
    return np.ascontiguousarray(
        xc.astype(MM_NP).reshape(NBT, BT, KCH, KP).transpose(3, 0, 2, 1)
    )


def kernel(x, conv_w, w1, b1, w2, b2):
    x = np.asarray(x, dtype=np.float32)
    w1e_r, b1_r, w2_r = _host_prep_weights(
        np.asarray(conv_w, np.float32),
        np.asarray(w1, np.float32),
        np.asarray(b1, np.float32),
        np.asarray(w2, np.float32),
    )
    b2 = np.asarray(b2, np.float32).reshape(1, O)

    if "nc" not in _cache:
        _cache["nc"] = _build_nc()
    nc = _cache["nc"]

    in_maps = []
    for c in range(N_CORES):
        xc = x[c * BS : (c + 1) * BS]  # [BS, 784]
        in_maps.append(
            {"xt": _host_prep_x(xc), "w1e": w1e_r, "b1r": b1_r, "w2r": w2_r}
        )

    res = run_bass_kernel_spmd(nc, in_maps, list(range(N_CORES)))

    y = np.empty((B, O), dtype=np.float32)
    for c in range(N_CORES):
        ytp = res.results[c]["ytp"]  # [YP_P, BS]
        if L2_COLTILE:
            yc = ytp[PB[0] : PB[0] + O]
            for j in range(1, MCH):
                yc = yc + ytp[PB[j] : PB[j] + O]
        else:
            yc = ytp[0:O]
        y[c * BS : (c + 1) * BS] = yc.T + b2
    return y
